# revision 3
# baseline (speedup 1.0000x reference)
"""Trainium2 Bass kernel for per-sample modulated/demodulated 3x3 conv.

Problem: x (8,512,32,32), s (8,512), w (512,512,3,3) ->
  wm[b,o,i,ky,kx] = w * (s[b,i]+1); demod by rsqrt(sum wm^2 + eps) per (b,o);
  y[b] = conv2d_same(x[b], wm[b]).

Sharding: data-parallel over batch, 1 sample per NeuronCore (8 cores).

Per-core algorithm (all fp32):
  - modulation folded into x: x'[i,p] = x[i,p] * (1+s_i)  (cheaper than
    scaling w: 1024 elems/chan vs 4608)
  - demodulation folded into output: y = (conv(x', w)) * denom[o], with
    denom[o] = 1/sqrt(sum_i (1+s_i)^2 * wsq[i,o] + eps),
    wsq[i,o] = sum_pos w[o,i,pos]^2 (computed on ACT/DVE, off critical path),
    and the contraction over i done with a tiny PE matvec.
  - conv as 9 shifted-window matmuls per (cin_chunk, cout_chunk) pair over a
    zero-padded 34x34 x buffer; accumulated in all 8 PSUM banks
    (4 cout chunks x 2 spatial halves of 512 pixels).

w is pre-packed host-side to w9[cin_chunk, 128, pos, cout] so the matmul
lhsT (contraction dim = cin on partitions) loads contiguously.
"""

import sys

if "/opt/trn_rl_repo" not in sys.path:
    sys.path.insert(0, "/opt/trn_rl_repo")

import numpy as np

B = 8
CIN = 512
COUT = 512
H = 32
W = 32
KPOS = 9  # 3x3 kernel positions
HP = H + 2  # padded
WP = W + 2
NCH = CIN // 128  # cin chunks
OCH = COUT // 128  # cout chunks
EPS = 1e-8

_compiled_nc = None


def _build():
    import concourse.tile as tile
    from concourse import bacc, mybir

    F32 = mybir.dt.float32

    nc = bacc.Bacc("TRN2", target_bir_lowering=False, debug=False, num_devices=B)
    x_d = nc.dram_tensor("x", [CIN, H, W], F32, kind="ExternalInput").ap()
    s_d = nc.dram_tensor("s", [CIN, 1], F32, kind="ExternalInput").ap()
    w9_d = nc.dram_tensor("w9", [NCH, 128, KPOS, COUT], F32, kind="ExternalInput").ap()
    y_d = nc.dram_tensor("y", [COUT, H * W], F32, kind="ExternalOutput").ap()

    with tile.TileContext(nc) as tc:
        with (
            tc.tile_pool(name="wpool", bufs=1) as wpool,
            tc.tile_pool(name="xpool", bufs=1) as xpool,
            tc.tile_pool(name="sqpool", bufs=2) as sqpool,
            tc.tile_pool(name="misc", bufs=1) as misc,
            tc.tile_pool(name="ypool", bufs=1) as ypool,
            tc.tile_pool(name="psum", bufs=8, space="PSUM") as psum,
        ):
            w_sb = [
                wpool.tile([128, KPOS, COUT], F32, name=f"w_sb{c}", tag=f"w{c}")
                for c in range(NCH)
            ]
            xp = [
                xpool.tile([128, HP, WP], F32, name=f"xp{c}", tag=f"x{c}")
                for c in range(NCH)
            ]
            s1 = [
                misc.tile([128, 1], F32, name=f"s1_{c}", tag=f"s1_{c}")
                for c in range(NCH)
            ]
            q = [
                misc.tile([128, 1], F32, name=f"q_{c}", tag=f"q_{c}")
                for c in range(NCH)
            ]
            wsq = [
                misc.tile([128, COUT], F32, name=f"wsq{c}", tag=f"wsq{c}")
                for c in range(NCH)
            ]
            den_s = [
                misc.tile([128, 1], F32, name=f"den_s{o}", tag=f"den_s{o}")
                for o in range(OCH)
            ]
            den = [
                misc.tile([128, 1], F32, name=f"den{o}", tag=f"den{o}")
                for o in range(OCH)
            ]
            y_sb = [
                ypool.tile([128, H * W], F32, name=f"y_sb{o}", tag=f"y{o}")
                for o in range(OCH)
            ]
            eps_t = misc.tile([128, 1], F32, name="eps_t", tag="eps_t")
            nc.vector.memset(eps_t, EPS)

            # --- input DMAs + x modulation; chunk-ordered so chunk 0 is ready
            # first and conv matmuls can start during the remaining loads.
            for c in range(NCH):
                nc.sync.dma_start(out=s1[c], in_=s_d[c * 128 : (c + 1) * 128, :])
                nc.scalar.add(s1[c], s1[c], 1.0)  # 1 + s
                nc.scalar.square(q[c], s1[c])  # (1 + s)^2

                xv = xp[c]
                nc.vector.memset(xv[:, 0, :], 0.0)
                nc.vector.memset(xv[:, HP - 1, :], 0.0)
                nc.vector.memset(xv[:, 1 : HP - 1, 0], 0.0)
                nc.vector.memset(xv[:, 1 : HP - 1, WP - 1], 0.0)
                nc.sync.dma_start(
                    out=xv[:, 1 : H + 1, 1 : W + 1],
                    in_=x_d[c * 128 : (c + 1) * 128, :, :],
                )
                nc.vector.tensor_scalar_mul(
                    xv[:, 1 : H + 1, 1 : W + 1], xv[:, 1 : H + 1, 1 : W + 1], s1[c]
                )
                # w chunk in 3 DMAs (3 kernel positions each) to shorten the
                # lead-in before the first matmul.
                for pg in range(3):
                    nc.sync.dma_start(
                        out=w_sb[c][:, pg * 3 : (pg + 1) * 3, :],
                        in_=w9_d[c, :, pg * 3 : (pg + 1) * 3, :],
                    )

            # --- conv: accumulate 36 matmuls into each of the 8 PSUM banks
            acc = [
                [
                    psum.tile([128, 512], F32, name=f"acc{o}_{hh}", tag="acc")
                    for hh in range(2)
                ]
                for o in range(OCH)
            ]
            for c in range(NCH):
                xv = xp[c]
                for pos in range(KPOS):
                    ky, kx = pos // 3, pos % 3
                    for o in range(OCH):
                        lhsT = w_sb[c][:, pos, o * 128 : (o + 1) * 128]
                        for hh in range(2):
                            rhs = xv[
                                :,
                                ky + hh * 16 : ky + hh * 16 + 16,
                                kx : kx + 32,
                            ]
                            nc.tensor.matmul(
                                acc[o][hh],
                                lhsT=lhsT,
                                rhs=rhs,
                                start=(c == 0 and pos == 0),
                                stop=(c == NCH - 1 and pos == KPOS - 1),
                            )

            # --- demod stats, all overlapped with the conv matmuls above:
            # wsq[i, o] = sum_pos w^2 (ACT square, DVE reduce over pos)
            for c in range(NCH):
                sq = sqpool.tile([128, KPOS, COUT], F32, name=f"sq{c}", tag="sq")
                nc.scalar.square(sq, w_sb[c])
                nc.vector.tensor_reduce(
                    out=wsq[c],
                    in_=sq.rearrange("p a b -> p b a"),
                    axis=mybir.AxisListType.X,
                    op=mybir.AluOpType.add,
                )

            # --- drain spatial-half 0 unscaled (frees 4 PSUM banks for the
            # demod matvec; scaled in-place once denom is ready)
            for o in range(OCH):
                nc.scalar.copy(y_sb[o][:, 0:512], acc[o][0])

            # denom[o] = 1/sqrt(sum_i q_i * wsq[i,o] + eps) via PE matvec
            dsum = [
                psum.tile([128, 1], F32, name=f"dsum{o}", tag="acc")
                for o in range(OCH)
            ]
            for o in range(OCH):
                for c in range(NCH):
                    nc.tensor.matmul(
                        dsum[o],
                        lhsT=wsq[c][:, o * 128 : (o + 1) * 128],
                        rhs=q[c],
                        start=(c == 0),
                        stop=(c == NCH - 1),
                    )
            for o in range(OCH):
                nc.scalar.activation(
                    den_s[o], dsum[o], mybir.ActivationFunctionType.Sqrt, bias=eps_t
                )
                nc.vector.reciprocal(den[o], den_s[o])

            # --- scaled drain of half 1, fix half 0, store
            for o in range(OCH):
                nc.vector.tensor_scalar_mul(y_sb[o][:, 512:1024], acc[o][1], den[o])
                nc.vector.tensor_scalar_mul(
                    y_sb[o][:, 0:512], y_sb[o][:, 0:512], den[o]
                )
                nc.sync.dma_start(
                    out=y_d[o * 128 : (o + 1) * 128, :], in_=y_sb[o]
                )

    nc.compile()
    return nc


def kernel(x, s, w):
    from concourse.bass_utils import run_bass_kernel_spmd

    global _compiled_nc
    if _compiled_nc is None:
        _compiled_nc = _build()
    nc = _compiled_nc

    x = np.asarray(x, dtype=np.float32)
    s = np.asarray(s, dtype=np.float32)
    w = np.asarray(w, dtype=np.float32)
    # w9[c, p, pos, o] = w[o, c*128+p, pos//3, pos%3]
    w9 = np.ascontiguousarray(np.transpose(w, (1, 2, 3, 0))).reshape(
        NCH, 128, KPOS, COUT
    )
    in_maps = [
        {
            "x": np.ascontiguousarray(x[i]),
            "s": np.ascontiguousarray(s[i].reshape(CIN, 1)),
            "w9": w9,
        }
        for i in range(B)
    ]
    res = run_bass_kernel_spmd(nc, in_maps, list(range(B))).results
    return np.stack([res[i]["y"].reshape(COUT, H, W) for i in range(B)], axis=0)


# revision 4
# speedup vs baseline: 2.8980x; 2.8980x over previous
"""Trainium2 Bass kernel for per-sample modulated/demodulated 3x3 conv.

Problem: x (8,512,32,32), s (8,512), w (512,512,3,3) ->
  wm[b,o,i,ky,kx] = w * (s[b,i]+1); demod by rsqrt(sum wm^2 + eps) per (b,o);
  y[b] = conv2d_same(x[b], wm[b]).

Sharding: data-parallel over batch, 1 sample per NeuronCore (8 cores).

Per-core algorithm:
  - modulation folded into x: x'[i,p] = x[i,p] * (1+s_i)  (cheaper than
    scaling w: 1024 elems/chan vs 4608)
  - demodulation folded into the output: y = conv(x', w) * denom[o], with
    denom[o] = 1/sqrt(sum_i (1+s_i)^2 * wsq[i,o] + eps),
    wsq[i,o] = sum_pos w[o,i,pos]^2 (DVE square + reduce, off critical path),
    contraction over i via a tiny PE matvec.
  - conv as 9 shifted-window matmuls per (cin_chunk, cout_chunk) pair over a
    zero-padded 34x34 x buffer; accumulated in all 8 PSUM banks
    (4 cout chunks x 2 spatial halves of 512 pixels).
  - matmuls run in bf16 (fp32 PE throughput is 1/4 of bf16 on TRN2); inputs
    arrive fp32 and are cast on device; PSUM accumulation is fp32; bf16*bf16
    products are exact in fp32, so the only error is input rounding (~1e-3).

w is pre-packed host-side to w9[cin_chunk, 128, pos, cout] so the matmul
lhsT (contraction dim = cin on partitions) loads contiguously.
"""

import os
import sys

if "/opt/trn_rl_repo" not in sys.path:
    sys.path.insert(0, "/opt/trn_rl_repo")

import numpy as np

B = 8
CIN = 512
COUT = 512
H = 32
W = 32
KPOS = 9  # 3x3 kernel positions
HP = H + 2  # padded
WP = W + 2
NCH = CIN // 128  # cin chunks
OCH = COUT // 128  # cout chunks
EPS = 1e-8

_compiled_nc = None


def _build():
    import concourse.tile as tile
    from concourse import bacc, mybir

    F32 = mybir.dt.float32
    MMDT = F32 if os.environ.get("BASS_CONV_F32") else mybir.dt.bfloat16

    nc = bacc.Bacc("TRN2", target_bir_lowering=False, debug=False, num_devices=B)
    x_d = nc.dram_tensor("x", [CIN, H, W], F32, kind="ExternalInput").ap()
    s_d = nc.dram_tensor("s", [CIN, 1], F32, kind="ExternalInput").ap()
    w9_d = nc.dram_tensor("w9", [NCH, 128, KPOS, COUT], F32, kind="ExternalInput").ap()
    y_d = nc.dram_tensor("y", [COUT, H * W], F32, kind="ExternalOutput").ap()

    with tile.TileContext(nc) as tc:
        with (
            tc.tile_pool(name="stage", bufs=2) as stage,
            tc.tile_pool(name="wpool", bufs=1) as wpool,
            tc.tile_pool(name="xpool", bufs=1) as xpool,
            tc.tile_pool(name="sqpool", bufs=2) as sqpool,
            tc.tile_pool(name="misc", bufs=1) as misc,
            tc.tile_pool(name="ypool", bufs=1) as ypool,
            tc.tile_pool(name="psum", bufs=8, space="PSUM") as psum,
        ):
            w_sb = [
                wpool.tile([128, KPOS, COUT], MMDT, name=f"w_sb{c}", tag=f"w{c}")
                for c in range(NCH)
            ]
            xp = [
                xpool.tile([128, HP, WP], MMDT, name=f"xp{c}", tag=f"x{c}")
                for c in range(NCH)
            ]
            s1 = [
                misc.tile([128, 1], F32, name=f"s1_{c}", tag=f"s1_{c}")
                for c in range(NCH)
            ]
            q = [
                misc.tile([128, 1], F32, name=f"q_{c}", tag=f"q_{c}")
                for c in range(NCH)
            ]
            wsq = [
                misc.tile([128, COUT], F32, name=f"wsq{c}", tag=f"wsq{c}")
                for c in range(NCH)
            ]
            den_s = [
                misc.tile([128, 1], F32, name=f"den_s{o}", tag=f"den_s{o}")
                for o in range(OCH)
            ]
            den = [
                misc.tile([128, 1], F32, name=f"den{o}", tag=f"den{o}")
                for o in range(OCH)
            ]
            y_sb = [
                ypool.tile([128, H * W], F32, name=f"y_sb{o}", tag=f"y{o}")
                for o in range(OCH)
            ]
            eps_t = misc.tile([128, 1], F32, name="eps_t", tag="eps_t")
            nc.vector.memset(eps_t, EPS)

            # --- input DMAs + casts + x modulation; chunk-ordered so chunk 0
            # is ready first and conv matmuls start during remaining loads.
            for c in range(NCH):
                nc.sync.dma_start(out=s1[c], in_=s_d[c * 128 : (c + 1) * 128, :])
                nc.scalar.add(s1[c], s1[c], 1.0)  # 1 + s
                nc.scalar.square(q[c], s1[c])  # (1 + s)^2

                xv = xp[c]
                nc.vector.memset(xv[:, 0, :], 0.0)
                nc.vector.memset(xv[:, HP - 1, :], 0.0)
                nc.vector.memset(xv[:, 1 : HP - 1, 0], 0.0)
                nc.vector.memset(xv[:, 1 : HP - 1, WP - 1], 0.0)
                xs = stage.tile([128, H, W], F32, name=f"xs{c}", tag="xstage")
                nc.sync.dma_start(out=xs, in_=x_d[c * 128 : (c + 1) * 128, :, :])
                # modulate + cast in one pass
                nc.vector.tensor_scalar_mul(xv[:, 1 : H + 1, 1 : W + 1], xs, s1[c])

                # w chunk: fp32 staging -> cast, in 3 pos-groups to shorten
                # the lead-in before the first matmul of this chunk.
                ws = stage.tile([128, KPOS, COUT], F32, name=f"ws{c}", tag="wstage")
                for pg in range(3):
                    sl = slice(pg * 3, (pg + 1) * 3)
                    nc.sync.dma_start(out=ws[:, sl, :], in_=w9_d[c, :, sl, :])
                    nc.scalar.copy(w_sb[c][:, sl, :], ws[:, sl, :])

                # demod stats (overlap with conv): wsq[i,o] = sum_pos w^2
                sq = sqpool.tile([128, KPOS, COUT], MMDT, name=f"sq{c}", tag="sq")
                nc.vector.tensor_mul(sq, w_sb[c], w_sb[c])
                nc.vector.tensor_reduce(
                    out=wsq[c],
                    in_=sq.rearrange("p a b -> p b a"),
                    axis=mybir.AxisListType.X,
                    op=mybir.AluOpType.add,
                )

            # --- conv: accumulate 36 matmuls into each of the 8 PSUM banks
            acc = [
                [
                    psum.tile([128, 512], F32, name=f"acc{o}_{hh}", tag="acc")
                    for hh in range(2)
                ]
                for o in range(OCH)
            ]
            for c in range(NCH):
                xv = xp[c]
                for pos in range(KPOS):
                    ky, kx = pos // 3, pos % 3
                    for o in range(OCH):
                        lhsT = w_sb[c][:, pos, o * 128 : (o + 1) * 128]
                        for hh in range(2):
                            rhs = xv[
                                :,
                                ky + hh * 16 : ky + hh * 16 + 16,
                                kx : kx + 32,
                            ]
                            nc.tensor.matmul(
                                acc[o][hh],
                                lhsT=lhsT,
                                rhs=rhs,
                                start=(c == 0 and pos == 0),
                                stop=(c == NCH - 1 and pos == KPOS - 1),
                            )

            # --- drain spatial-half 0 unscaled (frees 4 PSUM banks for the
            # demod matvec; scaled in-place once denom is ready)
            for o in range(OCH):
                nc.vector.tensor_copy(y_sb[o][:, 0:512], acc[o][0])

            # denom[o] = 1/sqrt(sum_i q_i * wsq[i,o] + eps) via PE matvec
            dsum = [
                psum.tile([128, 1], F32, name=f"dsum{o}", tag="acc")
                for o in range(OCH)
            ]
            for o in range(OCH):
                for c in range(NCH):
                    nc.tensor.matmul(
                        dsum[o],
                        lhsT=wsq[c][:, o * 128 : (o + 1) * 128],
                        rhs=q[c],
                        start=(c == 0),
                        stop=(c == NCH - 1),
                    )
            for o in range(OCH):
                nc.scalar.activation(
                    den_s[o], dsum[o], mybir.ActivationFunctionType.Sqrt, bias=eps_t
                )
                nc.vector.reciprocal(den[o], den_s[o])

            # --- scaled drain of half 1, fix half 0, store
            for o in range(OCH):
                nc.vector.tensor_scalar_mul(y_sb[o][:, 512:1024], acc[o][1], den[o])
                nc.vector.tensor_scalar_mul(
                    y_sb[o][:, 0:512], y_sb[o][:, 0:512], den[o]
                )
                nc.sync.dma_start(
                    out=y_d[o * 128 : (o + 1) * 128, :], in_=y_sb[o]
                )

    nc.compile()
    return nc


def kernel(x, s, w):
    from concourse.bass_utils import run_bass_kernel_spmd

    global _compiled_nc
    if _compiled_nc is None:
        _compiled_nc = _build()
    nc = _compiled_nc

    x = np.asarray(x, dtype=np.float32)
    s = np.asarray(s, dtype=np.float32)
    w = np.asarray(w, dtype=np.float32)
    # w9[c, p, pos, o] = w[o, c*128+p, pos//3, pos%3]
    w9 = np.ascontiguousarray(np.transpose(w, (1, 2, 3, 0))).reshape(
        NCH, 128, KPOS, COUT
    )
    in_maps = [
        {
            "x": np.ascontiguousarray(x[i]),
            "s": np.ascontiguousarray(s[i].reshape(CIN, 1)),
            "w9": w9,
        }
        for i in range(B)
    ]
    res = run_bass_kernel_spmd(nc, in_maps, list(range(B))).results
    return np.stack([res[i]["y"].reshape(COUT, H, W) for i in range(B)], axis=0)
